# revision 19
# baseline (speedup 1.0000x reference)
"""Trainium2 Bass kernel for nn_Decoder: autoregressive GRU -> LSTM -> Linear.

Strategy:
  - Data-parallel over 8 NeuronCores: batch 128 -> 16 per core; weights replicated.
  - Per core, gates-on-partition layout: weights (bf16) are the stationary matmul
    operand, per-step state (16 batch cols) streams as the moving operand.
  - The autoregressive GRU has x_prev == h from step 2 on, so the two gate matmuls
    collapse into one combined weight matrix [Wr_i+Wr_h; Wz_i+Wz_h; Wn_i; Wn_h].
  - Both recurrences are contractions for these weight scales: the hidden state
    converges to a fixed point to ~1 ulp after a few hundred steps.  The host
    detects the convergence step from the actual inputs (cheap numpy sim) and the
    device only computes K1 GRU / K2 LSTM steps; the remaining output rows are
    broadcast.  If the inputs do not converge, K1/K2 fall back to full length.
  - Device output layout is (D, T, 16) per core; host transposes/gathers.
"""

import math

import numpy as np
import ml_dtypes

B, T, L, H, D = 128, 1024, 256, 512, 30
NCORES = 8
BS = B // NCORES  # 16 batch per core

fp16 = np.float16

_NC_CACHE = {}


# ----------------------------------------------------------------------------
# Host-side numpy model (for convergence detection)
# ----------------------------------------------------------------------------

def _sigmoid(x):
    return 1.0 / (1.0 + np.exp(-x))


def _detect_K(z, gWi, gWh, gbi, gbh, lWi, lWh, lbi, lbh, n_steps):
    """Return (K1, K2): number of GRU steps (h_t computed for t=1..K1) and LSTM
    steps (t=0..K2-1) to run on device; output rows t>=K2 are broadcast."""
    f32 = np.float32
    tol = 1e-6

    x = np.zeros_like(z)
    h = z.astype(f32)
    gWiT, gWhT = gWi.T.astype(f32), gWh.T.astype(f32)
    k1 = n_steps - 1
    for t in range(1, n_steps):
        gi = (x @ gWiT + gbi).astype(f32)
        gh = (h @ gWhT + gbh).astype(f32)
        ir, iz, inn = np.split(gi, 3, -1)
        hr, hz, hn = np.split(gh, 3, -1)
        r = _sigmoid(ir + hr).astype(f32)
        zg = _sigmoid(iz + hz).astype(f32)
        n = np.tanh(inn + r * hn).astype(f32)
        hnew = ((1.0 - zg) * n + zg * h).astype(f32)
        d = float(np.max(np.abs(hnew - h)))
        x = hnew
        h = hnew
        if d < tol:
            k1 = t
            break
    # margin + alignment
    K1 = min(k1 + 15, n_steps - 1)
    K1 = 1 + 8 * int(math.ceil((K1 - 1) / 8))
    K1 = min(K1, n_steps - 1)

    # LSTM: input is relu(h_t) (constant for t>K1 up to tol); iterate until the
    # output stops changing.
    xin = np.maximum(h, 0.0)
    lWiT, lWhT = lWi.T.astype(f32), lWh.T.astype(f32)
    hh = np.zeros((z.shape[0], H), f32)
    c = np.zeros((z.shape[0], H), f32)
    k2 = n_steps
    for t in range(n_steps):
        g = (xin @ lWiT + lbi + hh @ lWhT + lbh).astype(f32)
        i, fg, gc, o = np.split(g, 4, -1)
        c = (_sigmoid(fg) * c + _sigmoid(i) * np.tanh(gc)).astype(f32)
        hprev = hh
        hh = (_sigmoid(o) * np.tanh(c)).astype(f32)
        if t > k1 + 2 and float(np.max(np.abs(hh - hprev))) < tol:
            k2 = t + 1
            break
    K2 = min(max(k2 + 24, K1 + 48), n_steps)
    K2 = 32 * int(math.ceil(K2 / 32))
    K2 = min(K2, n_steps)
    return K1, K2


# ----------------------------------------------------------------------------
# Device input prep (host precompute; replicated across cores)
# ----------------------------------------------------------------------------

def _prep_shared(gWi, gWh, gbi, gbh, lWi, lWh, lbi, lbh, fw, fb):
    f32 = np.float32
    # GRU combined weights, gate order [r z n m], m = hn-side
    Wc = np.concatenate([
        gWi[0:256] + gWh[0:256],
        gWi[256:512] + gWh[256:512],
        gWi[512:768],
        gWh[512:768],
    ], axis=0)  # (1024, 256)
    W1 = np.concatenate([
        gWh[0:256],
        gWh[256:512],
        np.zeros((256, 256), f32),
        gWh[512:768],
    ], axis=0)
    bias_g_vec = np.concatenate([
        gbi[0:256] + gbh[0:256],
        gbi[256:512] + gbh[256:512],
        gbi[512:768],
        gbh[512:768],
    ], axis=0)  # (1024,)

    def pack_lhsT(Wmat):
        # (M, K) weights -> lhsT (K, M) -> SBUF (128, K//128, M) bf16
        WT = Wmat.T.astype(fp16)  # (K, M)
        K = WT.shape[0]
        return np.ascontiguousarray(
            WT.reshape(K // 128, 128, WT.shape[1]).transpose(1, 0, 2))

    def bias_bcast(vec):
        # (M,) -> (128, (M//128)*16) f32: col block j = bias[128j..], bcast over batch
        M = vec.shape[0]
        bt = vec.reshape(M // 128, 128).T.astype(f32)           # (128, M//128)
        return np.ascontiguousarray(np.repeat(bt, BS, axis=1))  # (128, (M//128)*16)

    return {
        "wg": pack_lhsT(Wc),            # (128, 2, 1024) bf16
        "wg1": pack_lhsT(W1),           # (128, 2, 1024) bf16
        "bgt": bias_bcast(bias_g_vec),  # (128, 128) f32
        "wx": pack_lhsT(lWi),           # (128, 2, 2048) bf16
        "wh": pack_lhsT(lWh),           # (128, 4, 2048) bf16
        "bl": np.ascontiguousarray(
            (lbi + lbh).astype(f32).reshape(16, 128).T),  # (128, 16) f32
        "wf": pack_lhsT(fw),            # (128, 4, 30) bf16
        "fb": fb.astype(f32).reshape(D, 1),
    }


# ----------------------------------------------------------------------------
# Device program
# ----------------------------------------------------------------------------

def _build_program(K1, K2, n_steps, repeat=1):
    import concourse.bacc as bacc
    import concourse.bass as bass
    import concourse.mybir as mybir
    import concourse.tile as tile

    dt = mybir.dt

    nc = bacc.Bacc("TRN2", target_bir_lowering=False, debug=False, num_devices=NCORES)

    wg = nc.dram_tensor("wg", [128, 2, 1024], dt.float16, kind="ExternalInput").ap()
    wg1 = nc.dram_tensor("wg1", [128, 2, 1024], dt.float16, kind="ExternalInput").ap()
    bgt = nc.dram_tensor("bgt", [128, 128], dt.float32, kind="ExternalInput").ap()
    wx = nc.dram_tensor("wx", [128, 2, 2048], dt.float16, kind="ExternalInput").ap()
    wh = nc.dram_tensor("wh", [128, 4, 2048], dt.float16, kind="ExternalInput").ap()
    blt = nc.dram_tensor("bl", [128, 16], dt.float32, kind="ExternalInput").ap()
    wf = nc.dram_tensor("wf", [128, 4, 30], dt.float16, kind="ExternalInput").ap()
    fbt = nc.dram_tensor("fb", [D, 1], dt.float32, kind="ExternalInput").ap()
    h0 = nc.dram_tensor("h0", [128, 2, 16], dt.float32, kind="ExternalInput").ap()
    y = nc.dram_tensor("y", [D, n_steps, BS], dt.float32, kind="ExternalOutput").ap()

    with tile.TileContext(nc) as tc:
        if repeat == 1:
            _emit_body(nc, tc, bass, mybir, K1, K2, n_steps,
                       wg, wg1, bgt, wx, wh, blt, wf, fbt, h0, y)
        else:
            with tc.For_i(0, repeat, 1):
                _emit_body(nc, tc, bass, mybir, K1, K2, n_steps,
                           wg, wg1, bgt, wx, wh, blt, wf, fbt, h0, y)
    nc.compile()
    return nc


def _emit_body(nc, tc, bass, mybir, K1, K2, n_steps,
               wg, wg1, bgt_d, wx, wh, blt_d, wf, fbt, h0, y):
    import contextlib

    dt = mybir.dt
    AF = mybir.ActivationFunctionType
    f32, bf = dt.float32, dt.float16
    NSEQ = K2  # seq blocks t = 0..K2-1 (tail blocks filled by copy)

    ctx = contextlib.ExitStack()
    with ctx:
        singles = ctx.enter_context(tc.tile_pool(name="singles", bufs=1))
        state = ctx.enter_context(tc.tile_pool(name="state", bufs=1))
        work = ctx.enter_context(tc.tile_pool(name="work", bufs=3))
        psum = ctx.enter_context(tc.tile_pool(name="psum", bufs=2, space="PSUM"))
        psg = ctx.enter_context(tc.tile_pool(name="psg", bufs=1, space="PSUM"))
        psfc = ctx.enter_context(tc.tile_pool(name="psfc", bufs=1, space="PSUM"))

        # ---- load constants ----
        wg_sb = singles.tile([128, 2, 1024], bf)
        wg1_sb = singles.tile([128, 2, 1024], bf)
        bgt = singles.tile([128, 128], f32)
        wx_sb = singles.tile([128, 2, 2048], bf)
        wh_sb = singles.tile([128, 4, 2048], bf)
        blt = singles.tile([128, 16], f32)
        wf_sb = singles.tile([128, 4, 30], bf)
        fb_sb = singles.tile([D, 1], f32)
        h0_sb = singles.tile([128, 2, 16], f32)
        nc.sync.dma_start(out=wg_sb, in_=wg)
        nc.sync.dma_start(out=wg1_sb, in_=wg1)
        nc.sync.dma_start(out=bgt, in_=bgt_d)
        nc.sync.dma_start(out=wx_sb, in_=wx)
        nc.sync.dma_start(out=wh_sb, in_=wh)
        nc.sync.dma_start(out=blt, in_=blt_d)
        nc.sync.dma_start(out=wf_sb, in_=wf)
        nc.sync.dma_start(out=fb_sb, in_=fbt)
        nc.sync.dma_start(out=h0_sb, in_=h0)

        # ---- persistent state ----
        seq_sb = state.tile([128, NSEQ * 32], bf)      # relu(h_t) transposed
        hso = state.tile([128, 2, 32], f32)            # GRU h (f32), ping-pong
        hsb = state.tile([128, 2, 32], bf)             # GRU h (bf16)
        hb0 = state.tile([128, 2, 16], bf)             # h0 in bf16
        xs_sb = state.tile([D, K2 * 16], f32)          # FC outputs, (30, t*16+b)
        cst = state.tile([128, 2, 64], f32)            # LSTM c, ping-pong
        hlb = state.tile([128, 2, 64], bf)             # LSTM h (bf16), ping-pong

        nc.vector.memset(seq_sb[:, 0:32], 0.0)         # seq_0 = relu(0) = 0
        nc.vector.memset(cst[:, 0, :], 0.0)
        nc.vector.memset(hlb[:, 0, :], 0.0)
        nc.vector.tensor_copy(out=hb0[:], in_=h0_sb[:])

        # ================= GRU =================
        # h_t lives in buffer t%2; step t consumes h_{t-1}, produces h_t.
        def gru_step(w_sb, rhs_tiles, hprev_f32, t):
            dst = t % 2
            G = psg.tile([128, 128], f32, tag="gru_ps")
            for j in range(8):
                for k in range(2):
                    nc.tensor.matmul(
                        G[:, j * 16:(j + 1) * 16],
                        w_sb[:, k, 128 * j:128 * (j + 1)],
                        rhs_tiles[k],
                        start=(k == 0), stop=(k == 1),
                    )
            nc.vector.tensor_add(out=G[:], in0=G[:], in1=bgt[:])
            S = work.tile([128, 64], f32, tag="gru_s")
            nc.scalar.activation(S[:], G[:, 0:64], AF.Sigmoid)
            rh = work.tile([128, 32], f32, tag="gru_rh")
            nc.vector.tensor_mul(out=rh, in0=S[:, 0:32], in1=G[:, 96:128])
            nin = work.tile([128, 32], f32, tag="gru_nin")
            nc.vector.tensor_add(out=nin, in0=G[:, 64:96], in1=rh)
            N = work.tile([128, 32], f32, tag="gru_n")
            nc.scalar.activation(N[:], nin[:], AF.Tanh)
            dd = work.tile([128, 32], f32, tag="gru_d")
            nc.vector.tensor_sub(out=dd, in0=hprev_f32, in1=N[:])
            ee = work.tile([128, 32], f32, tag="gru_e")
            nc.vector.tensor_mul(out=ee, in0=S[:, 32:64], in1=dd[:])
            nc.vector.tensor_add(out=hso[:, dst, :], in0=ee[:], in1=N[:])
            nc.vector.tensor_copy(out=hsb[:, dst, :], in_=hso[:, dst, :])
            nc.scalar.activation(seq_sb[:, 32 * t:32 * t + 32],
                                 hso[:, dst, :], AF.Relu)

        def gru_step_n(t):
            if t == 1:
                gru_step(wg1_sb, [hb0[:, 0, :], hb0[:, 1, :]],
                         h0_sb[:].rearrange("p k b -> p (k b)"), 1)
            else:
                src = (t - 1) % 2
                gru_step(wg_sb, [hsb[:, src, 0:16], hsb[:, src, 16:32]],
                         hso[:, src, :], t)

        def seq_tail_fill():
            # seq blocks K1+1 .. K2-1 = copy of block K1 (log doubling)
            nfill = K2 - (K1 + 1)
            src0 = 32 * K1
            filled = 0
            while filled < nfill:
                n = min(filled + 1, nfill - filled)
                nc.vector.tensor_copy(
                    out=seq_sb[:, src0 + 32 * (filled + 1):
                               src0 + 32 * (filled + 1 + n)],
                    in_=seq_sb[:, src0:src0 + 32 * n])
                filled += n

        # GRU head start: H steps before the LSTM begins; the rest interleave.
        HSTART = min(64, K1)
        for t in range(1, HSTART + 1):
            gru_step_n(t)
        if HSTART == K1:
            seq_tail_fill()

        # ================= LSTM + FC =================
        # Batched input-side gates: gi[t] = Wx @ seq_t + bias, computed in
        # chunks of TCH steps (double-buffered), overlapped with consumption.
        TCH = 32
        nchunks = (K2 + TCH - 1) // TCH
        gipool = ctx.enter_context(tc.tile_pool(name="gich", bufs=3))
        gips = ctx.enter_context(tc.tile_pool(name="gips", bufs=2, space="PSUM"))
        gi_tiles = {}

        def gen_gi(c):
            t0 = TCH * c
            nt = min(TCH, K2 - t0)
            gt = gipool.tile([128, 16, TCH * 16], f32, tag="gi")
            gi_tiles[c] = gt
            # rhs: seq cols for steps t0..t0+nt-1, K-tile k
            sv = seq_sb[:, 32 * t0:32 * (t0 + nt)].rearrange(
                "p (t k b) -> p t k b", k=2, b=16)
            for j in range(16):
                P = gips.tile([128, TCH * 16], f32, tag="gi_ps")
                for k in range(2):
                    nc.tensor.matmul(
                        P[:, 0:nt * 16].rearrange("p (t b) -> p t b", b=16),
                        wx_sb[:, k, 128 * j:128 * (j + 1)],
                        sv[:, :, k, :],
                        start=(k == 0), stop=(k == 1))
                nc.scalar.activation(gt[:, j, 0:nt * 16], P[:, 0:nt * 16],
                                     AF.Identity, bias=blt[:, j:j + 1])

        # state for step t is in buffer t%2 (t=0: zeros in buffer 0)
        def lstm_step(t):
            src, dst = t % 2, (t + 1) % 2
            c_idx, tloc = divmod(t, TCH)
            gi = gi_tiles[c_idx]
            Gif = psum.tile([128, 128], f32, tag="l_ps_if")
            Ggo = psum.tile([128, 128], f32, tag="l_ps_go")
            for j in range(16):
                G = Gif if j < 8 else Ggo
                jj = j % 8
                for k in range(4):
                    nc.tensor.matmul(
                        G[:, jj * 16:(jj + 1) * 16],
                        wh_sb[:, k, 128 * j:128 * (j + 1)],
                        hlb[:, src, 16 * k:16 * k + 16],
                        start=(k == 0), stop=(k == 3))
            nc.vector.tensor_add(
                out=Gif[:].rearrange("p (j b) -> p j b", b=16),
                in0=Gif[:].rearrange("p (j b) -> p j b", b=16),
                in1=gi[:, 0:8, 16 * tloc:16 * tloc + 16])
            nc.vector.tensor_add(
                out=Ggo[:].rearrange("p (j b) -> p j b", b=16),
                in0=Ggo[:].rearrange("p (j b) -> p j b", b=16),
                in1=gi[:, 8:16, 16 * tloc:16 * tloc + 16])
            Sif = work.tile([128, 128], f32, tag="l_sif")
            nc.scalar.activation(Sif[:], Gif[:], AF.Sigmoid)
            Tg = work.tile([128, 64], f32, tag="l_tg")
            nc.scalar.activation(Tg[:], Ggo[:, 0:64], AF.Tanh)
            So = work.tile([128, 64], f32, tag="l_so")
            nc.scalar.activation(So[:], Ggo[:, 64:128], AF.Sigmoid)
            a = work.tile([128, 64], f32, tag="l_a")
            nc.vector.tensor_mul(out=a, in0=Sif[:, 64:128], in1=cst[:, src, :])
            b2 = work.tile([128, 64], f32, tag="l_b")
            nc.vector.tensor_mul(out=b2, in0=Sif[:, 0:64], in1=Tg[:])
            nc.vector.tensor_add(out=cst[:, dst, :], in0=a[:], in1=b2[:])
            tcn = work.tile([128, 64], f32, tag="l_tc")
            nc.scalar.activation(tcn[:], cst[:, dst, :], AF.Tanh)
            nc.vector.tensor_mul(out=hlb[:, dst, :], in0=So[:], in1=tcn[:])
            # FC: xsT (30, 16) = fw @ h_t
            P = psfc.tile([D, 16], f32, tag="fc_ps")
            for k in range(4):
                nc.tensor.matmul(P[:], wf_sb[:, k, :],
                                 hlb[:, dst, 16 * k:16 * k + 16],
                                 start=(k == 0), stop=(k == 3))
            nc.scalar.activation(xs_sb[:, 16 * t:16 * t + 16], P[:],
                                 AF.Identity, bias=fb_sb[:])

        gen_gi(0)
        if nchunks > 1:
            gen_gi(1)
        for t in range(K2):
            lstm_step(t)
            g = t + HSTART + 1
            if g <= K1:
                gru_step_n(g)
                if g == K1:
                    seq_tail_fill()
            if t % TCH == TCH - 1 and t // TCH + 2 < nchunks:
                gen_gi(t // TCH + 2)

        # ---- output DMAs ----
        nchunk = 4
        ychunk = max(64, (K2 + nchunk - 1) // nchunk)
        t0 = 0
        while t0 < K2:
            t1 = min(t0 + ychunk, K2)
            nc.sync.dma_start(
                out=y[:, t0:t1, :],
                in_=xs_sb[:, 16 * t0:16 * t1].rearrange("d (t b) -> d t b", b=BS))
            t0 = t1
        # tail: rows K2..n_steps-1 = row K2-1
        ntail = n_steps - K2
        if ntail > 0:
            tail64 = state.tile([D, 64 * 16], f32)
            nc.vector.tensor_copy(out=tail64[:, 0:16],
                                  in_=xs_sb[:, 16 * (K2 - 1):16 * K2])
            filled = 1
            while filled < 64:
                n = min(filled, 64 - filled)
                nc.vector.tensor_copy(
                    out=tail64[:, 16 * filled:16 * (filled + n)],
                    in_=tail64[:, 0:16 * n])
                filled += n
            t0 = K2
            while t0 < n_steps:
                n = min(64, n_steps - t0)
                nc.sync.dma_start(
                    out=y[:, t0:t0 + n, :],
                    in_=tail64[:, 0:16 * n].rearrange("d (t b) -> d t b", b=BS))
                t0 += n


# ----------------------------------------------------------------------------
# Public entry
# ----------------------------------------------------------------------------

def _get_program(K1, K2, n_steps):
    key = (K1, K2, n_steps)
    if key not in _NC_CACHE:
        _NC_CACHE[key] = _build_program(K1, K2, n_steps)
    return _NC_CACHE[key]


def _run(nc, in_maps):
    from concourse.bass_utils import run_bass_kernel_spmd
    return run_bass_kernel_spmd(nc, in_maps, core_ids=list(range(NCORES)))


def _make_in_maps(z, shared):
    in_maps = []
    for c in range(NCORES):
        zs = z[c * BS:(c + 1) * BS]  # (16, 256)
        h0c = np.ascontiguousarray(
            zs.T.reshape(2, 128, BS).transpose(1, 0, 2)).astype(np.float32)
        m = dict(shared)
        m["h0"] = h0c
        in_maps.append(m)
    return in_maps


def kernel(z, batch_sequence_length, gru_w_ih, gru_w_hh, gru_b_ih, gru_b_hh,
           lstm_w_ih, lstm_w_hh, lstm_b_ih, lstm_b_hh, fc_w, fc_b,
           _K_override=None):
    n_steps = int(batch_sequence_length)
    z = np.asarray(z, np.float32)
    args = [np.asarray(a, np.float32) for a in
            (gru_w_ih, gru_w_hh, gru_b_ih, gru_b_hh,
             lstm_w_ih, lstm_w_hh, lstm_b_ih, lstm_b_hh, fc_w, fc_b)]
    (gWi, gWh, gbi, gbh, lWi, lWh, lbi, lbh, fw, fb) = args

    if _K_override is not None:
        K1, K2 = _K_override
    else:
        K1, K2 = _detect_K(z, gWi, gWh, gbi, gbh, lWi, lWh, lbi, lbh, n_steps)

    shared = _prep_shared(gWi, gWh, gbi, gbh, lWi, lWh, lbi, lbh, fw, fb)
    nc = _get_program(K1, K2, n_steps)
    res = _run(nc, _make_in_maps(z, shared))
    out = np.empty((B, n_steps, D), np.float32)
    for c in range(NCORES):
        out[c * BS:(c + 1) * BS] = res.results[c]["y"].transpose(2, 1, 0)
    return out
